# revision 8
# baseline (speedup 1.0000x reference)
"""Masked dot-product attention (ESIM masked_softmax) Trainium2 Bass kernel.

Math (per batch):
    s   = q @ k^T ; t = s * m  (== q @ (k*m)^T, exact since m is 0/1)
    p   = exp(t) * m / sum_k(exp(t) * m)   (max-subtraction cancels; |s|<~50
                                            so exp() stays in fp32 range)
    out = p @ v = (exp(t) @ [v*m | m]) -> numerator | denominator

Device mapping (per core, 2 batches, data-parallel over 8 cores):
  - scores are computed TRANSPOSED (k on partitions, q free) so exp(s^T) is
    directly the lhsT of the PV matmul; no O(Lq*Lk) transposes.
  - k*m / q are PE-transposed once per batch ([128,128] fp32 tiles), with q
    duplicated into both partition halves and k-blocks packed in pairs so the
    K=64 score matmuls row-tile two-at-a-time (measured 238ns / pair of
    N=512 bf16 matmuls).
  - S matmul runs as 3 bf16 passes over hi/lo split operands
    (qh*kh + qh*kl + ql*kh): within ~2^-16 of a full fp32 matmul, at bf16
    speed with LDWEIGHTS hidden.  ATT_S_MODE=f32r selects a single fp22 pass.
  - PV uses float32r (fp22) with stationary [v*m | m]: column 64 of the
    accumulated output is the softmax denominator for free.
  - out^T [65, Lq] is PE-transposed back in 128-column chunks and normalized
    with a per-partition reciprocal multiply.
"""

import os
import sys

import numpy as np

sys.path.insert(0, "/opt/trn_rl_repo")

import concourse.bacc as bacc
import concourse.bass as bass
import concourse.mybir as mybir
import concourse.tile as tile
from concourse import bass_utils
from concourse.masks import make_identity

B, LQ, LK, D = 16, 2048, 2048, 64
NCORES = 8
PB = B // NCORES  # batches per core
P = 128
NKB = LK // P  # 16 k-blocks
NQB = LQ // P  # 16 q-blocks

S_MODE = os.environ.get("ATT_S_MODE", "bf16_3p")  # "bf16_3p" | "f32r"
PV_MODE = os.environ.get("ATT_PV_MODE", "f32r")  # "f32r" | "fp32"

F32 = mybir.dt.float32
F32R = mybir.dt.float32r
BF16 = mybir.dt.bfloat16
EXP = mybir.ActivationFunctionType.Exp


class _BatchCtx:
    pass


def _attention_core(tc, q_d, k_d, v_d, m_d, o_d):
    """Emit the per-core program. All dram handles are per-core shards."""
    nc = tc.nc
    pools = []

    def pool(name, bufs, space="SBUF"):
        p = tc.alloc_tile_pool(name=name, bufs=bufs, space=space)
        pools.append(p)
        return p

    singles = pool("singles", 1)
    stage = pool("stage", 2)
    main = pool("main", 2)
    wtp = pool("wt", 8)
    outp = pool("outp", 2)
    smalls = pool("smalls", 4)

    ps_s = pool("ps_s", 2, space="PSUM")  # 2 x [128,1024] = 4 banks
    ps_pv = pool("ps_pv", 1, space="PSUM")  # 1 x [65,1024] = 2 banks
    ps_nat = pool("ps_nat", 2, space="PSUM")  # 2 x [128,65] = 2 banks

    ident = singles.tile([P, P], F32, tag="ident")
    make_identity(nc, ident)

    three = S_MODE == "bf16_3p"
    sdt = F32 if three else F32R

    def prep_io(b):
        bc = _BatchCtx()
        bc.m_sb = stage.tile([P, NKB], F32, tag="m", name=f"m_sb{b}")
        nc.sync.dma_start(out=bc.m_sb, in_=m_d[b].rearrange("(t p) -> p t", p=P))
        bc.knat = stage.tile([P, NKB, D], F32, tag="knat", name=f"knat{b}")
        nc.gpsimd.dma_start(out=bc.knat, in_=k_d[b].rearrange("(t p) d -> p t d", p=P))
        bc.qdup = stage.tile([P, NQB, 2, D], F32, tag="qdup", name=f"qdup{b}")
        qsrc = q_d[b].rearrange("(t p) d -> p t d", p=P)
        nc.sync.dma_start(out=bc.qdup[:, :, 0, :], in_=qsrc)
        nc.sync.dma_start(out=bc.qdup[:, :, 1, :], in_=qsrc)
        bc.vnat = stage.tile([P, NKB, D], F32, tag="vnat", name=f"vnat{b}")
        nc.gpsimd.dma_start(out=bc.vnat, in_=v_d[b].rearrange("(t p) d -> p t d", p=P))
        return bc

    def prep_compute(b, bc):
        # mask-folded k and the transposed/packed/split S operands.
        km = stage.tile([P, NKB, D], F32, tag="km", name=f"km{b}")
        for t in range(NKB):
            nc.vector.tensor_scalar_mul(
                km[:, t, :], bc.knat[:, t, :], bc.m_sb[:, t : t + 1]
            )
        bc.kmT = main.tile([P, NKB // 2, P], sdt, tag="kmT", name=f"kmT{b}")
        trk = ps_s.tile([P, 8 * P], F32, tag="s", name=f"trk{b}")
        for j in range(NKB // 2):
            nc.tensor.transpose(
                trk[:, j * P : (j + 1) * P], km[:, 2 * j : 2 * j + 2, :], ident
            )
        kmT_f = bc.kmT.rearrange("p a b -> p (a b)")
        nc.vector.tensor_copy(kmT_f, trk)
        if three:
            bc.kmTh = main.tile([P, NKB // 2, P], BF16, tag="kmTh", name=f"kmTh{b}")
            bc.kmTl = main.tile([P, NKB // 2, P], BF16, tag="kmTl", name=f"kmTl{b}")
            kmTh_f = bc.kmTh.rearrange("p a b -> p (a b)")
            nc.vector.tensor_copy(kmTh_f, kmT_f)
            nc.vector.tensor_sub(
                bc.kmTl.rearrange("p a b -> p (a b)"), kmT_f, kmTh_f
            )

        bc.qT = main.tile([P, LQ], sdt, tag="qT", name=f"qT{b}")
        if three:
            bc.qTh = main.tile([P, LQ], BF16, tag="qTh", name=f"qTh{b}")
            bc.qTl = main.tile([P, LQ], BF16, tag="qTl", name=f"qTl{b}")
        # per-1024-half so the h=0 stream can start before half 1 lands
        for g in range(2):
            tr = ps_s.tile([P, 8 * P], F32, tag="s", name=f"trq{b}_{g}")
            for i in range(8):
                t = g * 8 + i
                nc.tensor.transpose(tr[:, i * P : (i + 1) * P], bc.qdup[:, t], ident)
            half = slice(g * 8 * P, (g + 1) * 8 * P)
            nc.vector.tensor_copy(bc.qT[:, half], tr)
            if three:
                nc.vector.tensor_copy(bc.qTh[:, half], bc.qT[:, half])
                nc.vector.tensor_sub(
                    bc.qTl[:, half], bc.qT[:, half], bc.qTh[:, half]
                )

        # [v*m | m] stationary for PV (fp22-rounded when PV_MODE=f32r)
        bc.vme = stage.tile(
            [P, NKB, D + 1], F32R if PV_MODE == "f32r" else F32, tag="vme",
            name=f"vme{b}",
        )
        for t in range(NKB):
            nc.vector.tensor_scalar_mul(
                bc.vme[:, t, 0:D], bc.vnat[:, t, :], bc.m_sb[:, t : t + 1]
            )
        nc.vector.tensor_copy(bc.vme[:, :, D], bc.m_sb[:, :])
        bc.out_sb = outp.tile([P, NQB, D], F32, tag="osb", name=f"osb{b}")
        return bc

    def main_half(b, bc, h):
        pv = ps_pv.tile([65, 1024], F32, tag="pv", name=f"pv{b}_{h}")
        for j in range(NKB // 2):
            sA = ps_s.tile([P, 1024], F32, tag="s", name=f"sA{b}_{h}_{j}")
            sB = ps_s.tile([P, 1024], F32, tag="s", name=f"sB{b}_{h}_{j}")
            for c in range(2):
                qs = slice(h * 1024 + c * 512, h * 1024 + (c + 1) * 512)
                cs = slice(c * 512, (c + 1) * 512)
                if three:
                    passes = [
                        (bc.kmTh, bc.qTh, True, False),
                        (bc.kmTl, bc.qTh, False, False),
                        (bc.kmTh, bc.qTl, False, True),
                    ]
                else:
                    passes = [(bc.kmT, bc.qT, True, True)]
                for kt, qt, st, sp in passes:
                    nc.tensor.matmul(
                        sA[:, cs], kt[0:64, j, :], qt[0:64, qs],
                        start=st, stop=sp, tile_position=(0, 0),
                    )
                    nc.tensor.matmul(
                        sB[:, cs], kt[64:128, j, :], qt[64:128, qs],
                        start=st, stop=sp, tile_position=(64, 0),
                    )
            wdt = F32R if PV_MODE == "f32r" else F32
            wA = wtp.tile([P, 1024], wdt, tag="wt", name=f"wA{b}_{h}_{j}")
            wB = wtp.tile([P, 1024], wdt, tag="wt", name=f"wB{b}_{h}_{j}")
            nc.scalar.activation(out=wA, in_=sA, func=EXP)
            nc.scalar.activation(out=wB, in_=sB, func=EXP)
            for c in range(2):
                cs = slice(c * 512, (c + 1) * 512)
                for kb, w in ((2 * j, wA), (2 * j + 1, wB)):
                    nc.tensor.matmul(
                        pv[:, cs], bc.vme[:, kb, :], w[:, cs],
                        start=(kb == 0), stop=(kb == NKB - 1),
                    )
        # drain: transpose back to [q, 65] and normalize
        outT = outp.tile([D + 1, 1024], F32, tag="outT", name=f"outT{b}_{h}")
        nc.vector.tensor_copy(outT, pv)
        for qb in range(8):
            nat = ps_nat.tile([P, D + 1], F32, tag="nat", name=f"nat{b}_{h}_{qb}")
            nc.tensor.transpose(
                nat, outT[:, qb * P : (qb + 1) * P], ident[0 : D + 1, 0 : D + 1]
            )
            rc = smalls.tile([P, 1], F32, tag="rc", name=f"rc{b}_{h}_{qb}")
            nc.vector.reciprocal(rc, nat[:, D : D + 1])
            nc.vector.tensor_scalar_mul(
                bc.out_sb[:, h * 8 + qb, :], nat[:, 0:D], rc
            )

    def store(b, bc):
        nc.gpsimd.dma_start(
            out=o_d[b].rearrange("(t p) d -> p t d", p=P), in_=bc.out_sb
        )

    # Emission order interleaves batch 1's prep into batch 0's stream so the
    # PE never sees a batch boundary bubble.
    bcs = [prep_io(b) for b in range(PB)]
    prep_compute(0, bcs[0])
    main_half(0, bcs[0], 0)
    if PB > 1:
        prep_compute(1, bcs[1])
    main_half(0, bcs[0], 1)
    store(0, bcs[0])
    for b in range(1, PB):
        main_half(b, bcs[b], 0)
        main_half(b, bcs[b], 1)
        store(b, bcs[b])

    for p in reversed(pools):
        p.release()


_NC_CACHE = None


def _build_nc():
    global _NC_CACHE
    if _NC_CACHE is not None:
        return _NC_CACHE
    nc = bacc.Bacc(None, target_bir_lowering=False, debug=False)
    q_d = nc.dram_tensor("q", [PB, LQ, D], F32, kind="ExternalInput")
    k_d = nc.dram_tensor("k", [PB, LK, D], F32, kind="ExternalInput")
    v_d = nc.dram_tensor("v", [PB, LK, D], F32, kind="ExternalInput")
    m_d = nc.dram_tensor("m", [PB, LK], F32, kind="ExternalInput")
    o_d = nc.dram_tensor("out", [PB, LQ, D], F32, kind="ExternalOutput")
    with tile.TileContext(nc) as tc:
        _attention_core(tc, q_d, k_d, v_d, m_d, o_d)
    nc.compile()
    _NC_CACHE = nc
    return nc


def kernel(q, k, v, v_mask, _trace=False, _tmpdir=None):
    q = np.ascontiguousarray(q, dtype=np.float32)
    k = np.ascontiguousarray(k, dtype=np.float32)
    v = np.ascontiguousarray(v, dtype=np.float32)
    v_mask = np.ascontiguousarray(v_mask, dtype=np.float32)
    assert q.shape == (B, LQ, D), q.shape

    nc = _build_nc()
    in_maps = [
        {
            "q": q[i * PB : (i + 1) * PB],
            "k": k[i * PB : (i + 1) * PB],
            "v": v[i * PB : (i + 1) * PB],
            "m": v_mask[i * PB : (i + 1) * PB],
        }
        for i in range(NCORES)
    ]
    res = bass_utils.run_bass_kernel_spmd(
        nc, in_maps, core_ids=list(range(NCORES)), trace=_trace, tmpdir=_tmpdir
    )
    out = np.concatenate([r["out"] for r in res.results], axis=0)
    if _trace:
        kernel.last_results = res
    return out


# revision 11
# speedup vs baseline: 1.2735x; 1.2735x over previous
"""Masked dot-product attention (ESIM masked_softmax) Trainium2 Bass kernel.

Math (per batch):
    s   = q @ k^T ; t = s * m  (== q @ (k*m)^T, exact since m is 0/1)
    p   = exp(t) * m / sum_k(exp(t) * m)   (max-subtraction cancels; |s|<~50
                                            so exp() stays in fp32 range)
    out = p @ v = (exp(t) @ [v*m | m]) -> numerator | denominator

Device mapping (per core, 2 batches, data-parallel over 8 cores):
  - scores are computed TRANSPOSED (k on partitions, q free) so exp(s^T) is
    directly the lhsT of the PV matmul; no O(Lq*Lk) transposes.
  - k*m / q are PE-transposed once per batch ([128,128] fp32 tiles), with q
    duplicated into both partition halves and k-blocks packed in pairs so the
    K=64 score matmuls row-tile two-at-a-time (measured 238ns / pair of
    N=512 bf16 matmuls).
  - S matmul runs as 3 bf16 passes over hi/lo split operands
    (qh*kh + qh*kl + ql*kh): within ~2^-16 of a full fp32 matmul, at bf16
    speed with LDWEIGHTS hidden.  ATT_S_MODE=f32r selects a single fp22 pass.
  - PV uses float32r (fp22) with stationary [v*m | m]: column 64 of the
    accumulated output is the softmax denominator for free.
  - out^T [65, Lq] is PE-transposed back in 128-column chunks and normalized
    with a per-partition reciprocal multiply.
"""

import os
import sys

import numpy as np

sys.path.insert(0, "/opt/trn_rl_repo")

import concourse.bacc as bacc
import concourse.bass as bass
import concourse.mybir as mybir
import concourse.tile as tile
from concourse import bass_utils
from concourse.masks import make_identity

B, LQ, LK, D = 16, 2048, 2048, 64
NCORES = 8
PB = B // NCORES  # batches per core
P = 128
NKB = LK // P  # 16 k-blocks
NQB = LQ // P  # 16 q-blocks

S_MODE = os.environ.get("ATT_S_MODE", "bf16_3p")  # "bf16_3p" | "f32r"
PV_MODE = os.environ.get("ATT_PV_MODE", "f32r")  # "f32r" | "fp32"

F32 = mybir.dt.float32
F32R = mybir.dt.float32r
BF16 = mybir.dt.bfloat16
EXP = mybir.ActivationFunctionType.Exp


class _BatchCtx:
    pass


def _attention_core(tc, q_d, k_d, v_d, m_d, o_d):
    """Emit the per-core program. All dram handles are per-core shards."""
    nc = tc.nc
    pools = []

    def pool(name, bufs, space="SBUF"):
        p = tc.alloc_tile_pool(name=name, bufs=bufs, space=space)
        pools.append(p)
        return p

    singles = pool("singles", 1)
    stage = pool("stage", 2)
    main = pool("main", 2)
    wtp = pool("wt", 8)
    outp = pool("outp", 2)
    smalls = pool("smalls", 4)

    ps_s = pool("ps_s", 3, space="PSUM")  # 3 x [128,1024] = 6 banks
    ps_pv = pool("ps_pv", 2, space="PSUM")  # 2 x 1-bank slots (pv chunks + nat)

    ident = singles.tile([P, P], F32, tag="ident")
    make_identity(nc, ident)

    three = S_MODE == "bf16_3p"
    sdt = F32 if three else F32R

    def prep_io(b):
        bc = _BatchCtx()
        bc.m_sb = stage.tile([P, NKB], F32, tag="m", name=f"m_sb{b}")
        nc.sync.dma_start(out=bc.m_sb, in_=m_d[b].rearrange("(t p) -> p t", p=P))
        bc.knat = stage.tile([P, NKB, D], F32, tag="knat", name=f"knat{b}")
        nc.gpsimd.dma_start(out=bc.knat, in_=k_d[b].rearrange("(t p) d -> p t d", p=P))
        bc.qdup = stage.tile([P, NQB, 2, D], F32, tag="qdup", name=f"qdup{b}")
        qsrc = q_d[b].rearrange("(t p) d -> p t d", p=P)
        nc.sync.dma_start(out=bc.qdup[:, :, 0, :], in_=qsrc)
        nc.sync.dma_start(out=bc.qdup[:, :, 1, :], in_=qsrc)
        bc.vnat = stage.tile([P, NKB, D], F32, tag="vnat", name=f"vnat{b}")
        nc.gpsimd.dma_start(out=bc.vnat, in_=v_d[b].rearrange("(t p) d -> p t d", p=P))
        return bc

    def prep_compute(b, bc):
        # mask-folded k and the transposed/packed/split S operands.
        km = stage.tile([P, NKB, D], F32, tag="km", name=f"km{b}")
        for t in range(NKB):
            nc.vector.tensor_scalar_mul(
                km[:, t, :], bc.knat[:, t, :], bc.m_sb[:, t : t + 1]
            )
        bc.kmT = main.tile([P, NKB // 2, P], sdt, tag="kmT", name=f"kmT{b}")
        trk = ps_s.tile([P, 8 * P], F32, tag="s", name=f"trk{b}")
        for j in range(NKB // 2):
            nc.tensor.transpose(
                trk[:, j * P : (j + 1) * P], km[:, 2 * j : 2 * j + 2, :], ident
            )
        kmT_f = bc.kmT.rearrange("p a b -> p (a b)")
        nc.vector.tensor_copy(kmT_f, trk)
        if three:
            bc.kmTh = main.tile([P, NKB // 2, P], BF16, tag="kmTh", name=f"kmTh{b}")
            bc.kmTl = main.tile([P, NKB // 2, P], BF16, tag="kmTl", name=f"kmTl{b}")
            kmTh_f = bc.kmTh.rearrange("p a b -> p (a b)")
            nc.vector.tensor_copy(kmTh_f, kmT_f)
            nc.vector.tensor_sub(
                bc.kmTl.rearrange("p a b -> p (a b)"), kmT_f, kmTh_f
            )

        bc.qT = main.tile([P, LQ], sdt, tag="qT", name=f"qT{b}")
        if three:
            bc.qTh = main.tile([P, LQ], BF16, tag="qTh", name=f"qTh{b}")
            bc.qTl = main.tile([P, LQ], BF16, tag="qTl", name=f"qTl{b}")
        # per-1024-half so the h=0 stream can start before half 1 lands
        for g in range(2):
            tr = ps_s.tile([P, 8 * P], F32, tag="s", name=f"trq{b}_{g}")
            for i in range(8):
                t = g * 8 + i
                nc.tensor.transpose(tr[:, i * P : (i + 1) * P], bc.qdup[:, t], ident)
            half = slice(g * 8 * P, (g + 1) * 8 * P)
            nc.vector.tensor_copy(bc.qT[:, half], tr)
            if three:
                nc.vector.tensor_copy(bc.qTh[:, half], bc.qT[:, half])
                nc.vector.tensor_sub(
                    bc.qTl[:, half], bc.qT[:, half], bc.qTh[:, half]
                )

        # [v*m | m] stationary for PV (fp22-rounded when PV_MODE=f32r)
        bc.vme = stage.tile(
            [P, NKB, D + 1], F32R if PV_MODE == "f32r" else F32, tag="vme",
            name=f"vme{b}",
        )
        for t in range(NKB):
            nc.vector.tensor_scalar_mul(
                bc.vme[:, t, 0:D], bc.vnat[:, t, :], bc.m_sb[:, t : t + 1]
            )
        nc.vector.tensor_copy(bc.vme[:, :, D], bc.m_sb[:, :])
        bc.out_sb = outp.tile([P, NQB, D], F32, tag="osb", name=f"osb{b}")
        return bc

    def main_half(b, bc, h):
        pvc = [
            ps_pv.tile([65, 512], F32, tag="pv", name=f"pv{b}_{h}_{c}")
            for c in range(2)
        ]
        for j in range(NKB // 2):
            sA = ps_s.tile([P, 1024], F32, tag="s", name=f"sA{b}_{h}_{j}")
            sB = ps_s.tile([P, 1024], F32, tag="s", name=f"sB{b}_{h}_{j}")
            for c in range(2):
                qs = slice(h * 1024 + c * 512, h * 1024 + (c + 1) * 512)
                cs = slice(c * 512, (c + 1) * 512)
                if three:
                    passes = [
                        (bc.kmTh, bc.qTh, True, False),
                        (bc.kmTl, bc.qTh, False, False),
                        (bc.kmTh, bc.qTl, False, True),
                    ]
                else:
                    passes = [(bc.kmT, bc.qT, True, True)]
                for kt, qt, st, sp in passes:
                    nc.tensor.matmul(
                        sA[:, cs], kt[0:64, j, :], qt[0:64, qs],
                        start=st, stop=sp, tile_position=(0, 0),
                    )
                    nc.tensor.matmul(
                        sB[:, cs], kt[64:128, j, :], qt[64:128, qs],
                        start=st, stop=sp, tile_position=(64, 0),
                    )
            wdt = F32R if PV_MODE == "f32r" else F32
            wA = wtp.tile([P, 1024], wdt, tag="wt", name=f"wA{b}_{h}_{j}")
            wB = wtp.tile([P, 1024], wdt, tag="wt", name=f"wB{b}_{h}_{j}")
            nc.scalar.activation(out=wA, in_=sA, func=EXP)
            nc.scalar.activation(out=wB, in_=sB, func=EXP)
            for c in range(2):
                cs = slice(c * 512, (c + 1) * 512)
                for kb, w in ((2 * j, wA), (2 * j + 1, wB)):
                    nc.tensor.matmul(
                        pvc[c], bc.vme[:, kb, :], w[:, cs],
                        start=(kb == 0), stop=(kb == NKB - 1),
                    )
        # drain: transpose back to [q, 65] and normalize
        outT = outp.tile([D + 1, 1024], F32, tag="outT", name=f"outT{b}_{h}")
        for c in range(2):
            nc.vector.tensor_copy(outT[:, c * 512 : (c + 1) * 512], pvc[c])
        for qb in range(8):
            nat = ps_pv.tile([P, D + 1], F32, tag="pv", name=f"nat{b}_{h}_{qb}")
            nc.tensor.transpose(
                nat, outT[:, qb * P : (qb + 1) * P], ident[0 : D + 1, 0 : D + 1]
            )
            rc = smalls.tile([P, 1], F32, tag="rc", name=f"rc{b}_{h}_{qb}")
            nc.vector.reciprocal(rc, nat[:, D : D + 1])
            nc.vector.tensor_scalar_mul(
                bc.out_sb[:, h * 8 + qb, :], nat[:, 0:D], rc
            )

    def store(b, bc):
        nc.gpsimd.dma_start(
            out=o_d[b].rearrange("(t p) d -> p t d", p=P), in_=bc.out_sb
        )

    # Emission order interleaves batch 1's prep into batch 0's stream so the
    # PE never sees a batch boundary bubble.
    bcs = [prep_io(b) for b in range(PB)]
    prep_compute(0, bcs[0])
    main_half(0, bcs[0], 0)
    if PB > 1:
        prep_compute(1, bcs[1])
    main_half(0, bcs[0], 1)
    store(0, bcs[0])
    for b in range(1, PB):
        main_half(b, bcs[b], 0)
        main_half(b, bcs[b], 1)
        store(b, bcs[b])

    for p in reversed(pools):
        p.release()


_NC_CACHE = None


def _build_nc():
    global _NC_CACHE
    if _NC_CACHE is not None:
        return _NC_CACHE
    nc = bacc.Bacc(None, target_bir_lowering=False, debug=False)
    q_d = nc.dram_tensor("q", [PB, LQ, D], F32, kind="ExternalInput")
    k_d = nc.dram_tensor("k", [PB, LK, D], F32, kind="ExternalInput")
    v_d = nc.dram_tensor("v", [PB, LK, D], F32, kind="ExternalInput")
    m_d = nc.dram_tensor("m", [PB, LK], F32, kind="ExternalInput")
    o_d = nc.dram_tensor("out", [PB, LQ, D], F32, kind="ExternalOutput")
    with tile.TileContext(nc) as tc:
        _attention_core(tc, q_d, k_d, v_d, m_d, o_d)
    nc.compile()
    _NC_CACHE = nc
    return nc


def kernel(q, k, v, v_mask, _trace=False, _tmpdir=None):
    q = np.ascontiguousarray(q, dtype=np.float32)
    k = np.ascontiguousarray(k, dtype=np.float32)
    v = np.ascontiguousarray(v, dtype=np.float32)
    v_mask = np.ascontiguousarray(v_mask, dtype=np.float32)
    assert q.shape == (B, LQ, D), q.shape

    nc = _build_nc()
    in_maps = [
        {
            "q": q[i * PB : (i + 1) * PB],
            "k": k[i * PB : (i + 1) * PB],
            "v": v[i * PB : (i + 1) * PB],
            "m": v_mask[i * PB : (i + 1) * PB],
        }
        for i in range(NCORES)
    ]
    res = bass_utils.run_bass_kernel_spmd(
        nc, in_maps, core_ids=list(range(NCORES)), trace=_trace, tmpdir=_tmpdir
    )
    out = np.concatenate([r["out"] for r in res.results], axis=0)
    if _trace:
        kernel.last_results = res
    return out


# revision 12
# speedup vs baseline: 1.6489x; 1.2948x over previous
"""Masked dot-product attention (ESIM masked_softmax) Trainium2 Bass kernel.

Math (per batch):
    s   = q @ k^T ; t = s * m  (== q @ (k*m)^T, exact since m is 0/1)
    p   = exp(t) * m / sum_k(exp(t) * m)   (max-subtraction cancels; |s|<~50
                                            so exp() stays in fp32 range)
    out = p @ v = (exp(t) @ [v*m | m]) -> numerator | denominator

Device mapping (per core, 2 batches, data-parallel over 8 cores):
  - scores are computed TRANSPOSED (k on partitions, q free) so exp(s^T) is
    directly the lhsT of the PV matmul; no O(Lq*Lk) transposes.
  - k*m / q are PE-transposed once per batch ([128,128] fp32 tiles), with q
    duplicated into both partition halves and k-blocks packed in pairs so the
    K=64 score matmuls row-tile two-at-a-time (measured 238ns / pair of
    N=512 bf16 matmuls).
  - S matmul runs as 3 bf16 passes over hi/lo split operands
    (qh*kh + qh*kl + ql*kh): within ~2^-16 of a full fp32 matmul, at bf16
    speed with LDWEIGHTS hidden.  ATT_S_MODE=f32r selects a single fp22 pass.
  - PV uses float32r (fp22) with stationary [v*m | m]: column 64 of the
    accumulated output is the softmax denominator for free.
  - out^T [65, Lq] is PE-transposed back in 128-column chunks and normalized
    with a per-partition reciprocal multiply.
"""

import os
import sys

import numpy as np

sys.path.insert(0, "/opt/trn_rl_repo")

import concourse.bacc as bacc
import concourse.bass as bass
import concourse.mybir as mybir
import concourse.tile as tile
from concourse import bass_utils
from concourse.masks import make_identity

B, LQ, LK, D = 16, 2048, 2048, 64
NCORES = 8
PB = B // NCORES  # batches per core
P = 128
NKB = LK // P  # 16 k-blocks
NQB = LQ // P  # 16 q-blocks

S_MODE = os.environ.get("ATT_S_MODE", "bf16_3p")  # "bf16_3p" | "f32r"
PV_MODE = os.environ.get("ATT_PV_MODE", "f32r")  # "f32r" | "fp32"

F32 = mybir.dt.float32
F32R = mybir.dt.float32r
BF16 = mybir.dt.bfloat16
EXP = mybir.ActivationFunctionType.Exp


class _BatchCtx:
    pass


def _attention_core(tc, q_d, k_d, v_d, m_d, o_d):
    """Emit the per-core program. All dram handles are per-core shards."""
    nc = tc.nc
    pools = []

    def pool(name, bufs, space="SBUF"):
        p = tc.alloc_tile_pool(name=name, bufs=bufs, space=space)
        pools.append(p)
        return p

    singles = pool("singles", 1)
    stage = pool("stage", 2)
    main = pool("main", 2)
    wtp = pool("wt", 8)
    outp = pool("outp", 2)
    smalls = pool("smalls", 4)

    ps_s = pool("ps_s", 3, space="PSUM")  # 3 x [128,1024] = 6 banks
    ps_pv = pool("ps_pv", 2, space="PSUM")  # 2 x 1-bank slots (pv chunks + nat)

    ident = singles.tile([P, P], F32, tag="ident")
    make_identity(nc, ident)

    three = S_MODE == "bf16_3p"
    sdt = F32 if three else F32R

    def prep_io(b):
        bc = _BatchCtx()
        bc.m_sb = stage.tile([P, NKB], F32, tag="m", name=f"m_sb{b}")
        nc.sync.dma_start(out=bc.m_sb, in_=m_d[b].rearrange("(t p) -> p t", p=P))
        bc.knat = stage.tile([P, NKB, D], F32, tag="knat", name=f"knat{b}")
        nc.gpsimd.dma_start(out=bc.knat, in_=k_d[b].rearrange("(t p) d -> p t d", p=P))
        bc.qdup = stage.tile([P, NQB, 2, D], F32, tag="qdup", name=f"qdup{b}")
        qsrc = q_d[b].rearrange("(t p) d -> p t d", p=P)
        nc.sync.dma_start(out=bc.qdup[:, :, 0, :], in_=qsrc)
        nc.sync.dma_start(out=bc.qdup[:, :, 1, :], in_=qsrc)
        bc.vnat = stage.tile([P, NKB, D], F32, tag="vnat", name=f"vnat{b}")
        nc.gpsimd.dma_start(out=bc.vnat, in_=v_d[b].rearrange("(t p) d -> p t d", p=P))
        return bc

    def prep_compute(b, bc):
        # mask-folded k and the transposed/packed/split S operands.
        km = stage.tile([P, NKB, D], F32, tag="km", name=f"km{b}")
        for t in range(NKB):
            nc.vector.tensor_scalar_mul(
                km[:, t, :], bc.knat[:, t, :], bc.m_sb[:, t : t + 1]
            )
        bc.kmT = main.tile([P, NKB // 2, P], sdt, tag="kmT", name=f"kmT{b}")
        trk = ps_s.tile([P, 8 * P], F32, tag="s", name=f"trk{b}")
        for j in range(NKB // 2):
            nc.tensor.transpose(
                trk[:, j * P : (j + 1) * P], km[:, 2 * j : 2 * j + 2, :], ident
            )
        kmT_f = bc.kmT.rearrange("p a b -> p (a b)")
        nc.vector.tensor_copy(kmT_f, trk)
        if three:
            bc.kmTh = main.tile([P, NKB // 2, P], BF16, tag="kmTh", name=f"kmTh{b}")
            bc.kmTl = main.tile([P, NKB // 2, P], BF16, tag="kmTl", name=f"kmTl{b}")
            kmTh_f = bc.kmTh.rearrange("p a b -> p (a b)")
            nc.vector.tensor_copy(kmTh_f, kmT_f)
            nc.vector.tensor_sub(
                bc.kmTl.rearrange("p a b -> p (a b)"), kmT_f, kmTh_f
            )

        bc.qT = main.tile([P, LQ], sdt, tag="qT", name=f"qT{b}")
        if three:
            bc.qTh = main.tile([P, LQ], BF16, tag="qTh", name=f"qTh{b}")
            bc.qTl = main.tile([P, LQ], BF16, tag="qTl", name=f"qTl{b}")
        # per-1024-half so the h=0 stream can start before half 1 lands
        for g in range(2):
            tr = ps_s.tile([P, 8 * P], F32, tag="s", name=f"trq{b}_{g}")
            for i in range(8):
                t = g * 8 + i
                nc.tensor.transpose(tr[:, i * P : (i + 1) * P], bc.qdup[:, t], ident)
            half = slice(g * 8 * P, (g + 1) * 8 * P)
            nc.vector.tensor_copy(bc.qT[:, half], tr)
            if three:
                nc.vector.tensor_copy(bc.qTh[:, half], bc.qT[:, half])
                nc.vector.tensor_sub(
                    bc.qTl[:, half], bc.qT[:, half], bc.qTh[:, half]
                )

        # [v*m | m] stationary for PV (fp22-rounded when PV_MODE=f32r)
        bc.vme = stage.tile(
            [P, NKB, D + 1], F32R if PV_MODE == "f32r" else F32, tag="vme",
            name=f"vme{b}",
        )
        for t in range(NKB):
            nc.vector.tensor_scalar_mul(
                bc.vme[:, t, 0:D], bc.vnat[:, t, :], bc.m_sb[:, t : t + 1]
            )
        nc.vector.tensor_copy(bc.vme[:, :, D], bc.m_sb[:, :])
        bc.out_sb = outp.tile([P, NQB, D], F32, tag="osb", name=f"osb{b}")
        return bc

    def main_half(b, bc, h):
        pvc = [
            ps_pv.tile([65, 512], F32, tag="pv", name=f"pv{b}_{h}_{c}")
            for c in range(2)
        ]
        if three:
            passes = [
                (bc.kmTh, bc.qTh, True, False),
                (bc.kmTl, bc.qTh, False, False),
                (bc.kmTh, bc.qTl, False, True),
            ]
        else:
            passes = [(bc.kmT, bc.qT, True, True)]
        wdt = F32R if PV_MODE == "f32r" else F32

        def emit_pv(j, wA, wB):
            for c in range(2):
                cs = slice(c * 512, (c + 1) * 512)
                for kb, w in ((2 * j, wA), (2 * j + 1, wB)):
                    nc.tensor.matmul(
                        pvc[c], bc.vme[:, kb, :], w[:, cs],
                        start=(kb == 0), stop=(kb == NKB - 1),
                    )

        prev = None
        for j in range(NKB // 2):
            sA = ps_s.tile([P, 1024], F32, tag="s", name=f"sA{b}_{h}_{j}")
            sB = ps_s.tile([P, 1024], F32, tag="s", name=f"sB{b}_{h}_{j}")
            # c innermost: consecutive same-side matmuls alternate banks (no
            # accumulate drain-wait) and A/B stay adjacent so they row-pair.
            for kt, qt, st, sp in passes:
                for c in range(2):
                    qs = slice(h * 1024 + c * 512, h * 1024 + (c + 1) * 512)
                    cs = slice(c * 512, (c + 1) * 512)
                    nc.tensor.matmul(
                        sA[:, cs], kt[0:64, j, :], qt[0:64, qs],
                        start=st, stop=sp, tile_position=(0, 0),
                    )
                    nc.tensor.matmul(
                        sB[:, cs], kt[64:128, j, :], qt[64:128, qs],
                        start=st, stop=sp, tile_position=(64, 0),
                    )
            wA = wtp.tile([P, 1024], wdt, tag="wt", name=f"wA{b}_{h}_{j}")
            wB = wtp.tile([P, 1024], wdt, tag="wt", name=f"wB{b}_{h}_{j}")
            nc.scalar.activation(out=wA, in_=sA, func=EXP)
            nc.scalar.activation(out=wB, in_=sB, func=EXP)
            # PV for the previous j: its exp finished while S(j) streamed, so
            # the in-order PE never stalls on ScalarE here.
            if prev is not None:
                emit_pv(prev[0], prev[1], prev[2])
            prev = (j, wA, wB)
        emit_pv(*prev)
        # drain: transpose back to [q, 65] and normalize
        outT = outp.tile([D + 1, 1024], F32, tag="outT", name=f"outT{b}_{h}")
        for c in range(2):
            nc.vector.tensor_copy(outT[:, c * 512 : (c + 1) * 512], pvc[c])
        for qb in range(8):
            nat = ps_pv.tile([P, D + 1], F32, tag="pv", name=f"nat{b}_{h}_{qb}")
            nc.tensor.transpose(
                nat, outT[:, qb * P : (qb + 1) * P], ident[0 : D + 1, 0 : D + 1]
            )
            rc = smalls.tile([P, 1], F32, tag="rc", name=f"rc{b}_{h}_{qb}")
            nc.vector.reciprocal(rc, nat[:, D : D + 1])
            nc.vector.tensor_scalar_mul(
                bc.out_sb[:, h * 8 + qb, :], nat[:, 0:D], rc
            )

    def store(b, bc):
        nc.gpsimd.dma_start(
            out=o_d[b].rearrange("(t p) d -> p t d", p=P), in_=bc.out_sb
        )

    # Emission order interleaves batch 1's prep into batch 0's stream so the
    # PE never sees a batch boundary bubble.
    bcs = [prep_io(b) for b in range(PB)]
    prep_compute(0, bcs[0])
    main_half(0, bcs[0], 0)
    if PB > 1:
        prep_compute(1, bcs[1])
    main_half(0, bcs[0], 1)
    store(0, bcs[0])
    for b in range(1, PB):
        main_half(b, bcs[b], 0)
        main_half(b, bcs[b], 1)
        store(b, bcs[b])

    for p in reversed(pools):
        p.release()


_NC_CACHE = None


def _build_nc():
    global _NC_CACHE
    if _NC_CACHE is not None:
        return _NC_CACHE
    nc = bacc.Bacc(None, target_bir_lowering=False, debug=False)
    q_d = nc.dram_tensor("q", [PB, LQ, D], F32, kind="ExternalInput")
    k_d = nc.dram_tensor("k", [PB, LK, D], F32, kind="ExternalInput")
    v_d = nc.dram_tensor("v", [PB, LK, D], F32, kind="ExternalInput")
    m_d = nc.dram_tensor("m", [PB, LK], F32, kind="ExternalInput")
    o_d = nc.dram_tensor("out", [PB, LQ, D], F32, kind="ExternalOutput")
    with tile.TileContext(nc) as tc:
        _attention_core(tc, q_d, k_d, v_d, m_d, o_d)
    nc.compile()
    _NC_CACHE = nc
    return nc


def kernel(q, k, v, v_mask, _trace=False, _tmpdir=None):
    q = np.ascontiguousarray(q, dtype=np.float32)
    k = np.ascontiguousarray(k, dtype=np.float32)
    v = np.ascontiguousarray(v, dtype=np.float32)
    v_mask = np.ascontiguousarray(v_mask, dtype=np.float32)
    assert q.shape == (B, LQ, D), q.shape

    nc = _build_nc()
    in_maps = [
        {
            "q": q[i * PB : (i + 1) * PB],
            "k": k[i * PB : (i + 1) * PB],
            "v": v[i * PB : (i + 1) * PB],
            "m": v_mask[i * PB : (i + 1) * PB],
        }
        for i in range(NCORES)
    ]
    res = bass_utils.run_bass_kernel_spmd(
        nc, in_maps, core_ids=list(range(NCORES)), trace=_trace, tmpdir=_tmpdir
    )
    out = np.concatenate([r["out"] for r in res.results], axis=0)
    if _trace:
        kernel.last_results = res
    return out


# revision 15
# speedup vs baseline: 1.8949x; 1.1492x over previous
"""Masked dot-product attention (ESIM masked_softmax) Trainium2 Bass kernel.

Math (per batch):
    s   = q @ k^T ; t = s * m  (== q @ (k*m)^T, exact since m is 0/1)
    p   = exp(t) * m / sum_k(exp(t) * m)   (max-subtraction cancels; |s|<~50
                                            so exp() stays in fp32 range)
    out = p @ v = (exp(t) @ [v*m | m]) -> numerator | denominator

Device mapping (per core, 2 batches, data-parallel over 8 cores):
  - masked key rows are compacted away on the host (kept rows first, zero-mask
    padding to LKC=1792), shrinking every O(Lq*Lk) stage by ~12%.
  - scores are computed TRANSPOSED (k on partitions, q free) so exp(s^T) is
    directly the lhsT of the PV matmul; no O(Lq*Lk) transposes.
  - k*m / q are PE-transposed once per batch ([128,128] fp32 tiles), with q
    duplicated into both partition halves and k-blocks packed in pairs so the
    K=64 score matmuls row-tile two-at-a-time (~218ns per pair of N=512
    bf16 matmuls).
  - S matmul: 3 bf16 passes over hi/lo split operands (qh*kh + qh*kl + ql*kh)
    = within ~2^-16 of a full fp32 matmul at bf16 speed with LDWEIGHTS
    hidden. ATT_S_MODE=f32r selects a single fp22 pass instead.
  - PV uses float32r (fp22) with stationary [v*m | m]: column 64 of the
    accumulated output is the softmax denominator for free.
  - out^T [65, Lq] is PE-transposed back in 128-column chunks and normalized
    with a per-partition reciprocal multiply.
"""

import os
import sys

import numpy as np

sys.path.insert(0, "/opt/trn_rl_repo")

import concourse.bacc as bacc
import concourse.bass as bass
import concourse.mybir as mybir
import concourse.tile as tile
from concourse import bass_utils
from concourse.masks import make_identity

B, LQ, LK, D = 16, 2048, 2048, 64
NCORES = 8
PB = B // NCORES  # batches per core
P = 128
NQB = LQ // P  # 16 q-blocks

S_MODE = os.environ.get("ATT_S_MODE", "bf16_3p")  # "bf16_3p" | "f32r"
PV_MODE = os.environ.get("ATT_PV_MODE", "f32r")  # "f32r" | "fp32"
COMPACT = os.environ.get("ATT_COMPACT", "1") == "1"
LKC = 1792  # compacted key length (14 blocks); used when counts allow

F32 = mybir.dt.float32
F32R = mybir.dt.float32r
BF16 = mybir.dt.bfloat16
EXP = mybir.ActivationFunctionType.Exp


class _BatchCtx:
    pass


def _attention_core(tc, q_d, k_d, v_d, m_d, o_d, nkb):
    """Emit the per-core program. All dram handles are per-core shards."""
    nc = tc.nc
    npair = nkb // 2
    pools = []

    def pool(name, bufs, space="SBUF"):
        p = tc.alloc_tile_pool(name=name, bufs=bufs, space=space)
        pools.append(p)
        return p

    singles = pool("singles", 1)
    stage = pool("stage", 2)
    main = pool("main", 2)
    wtp = pool("wt", 8)
    outp = pool("outp", 2)
    smalls = pool("smalls", 4)

    ps_s = pool("ps_s", 3, space="PSUM")  # 3 x [128,1024] = 6 banks
    ps_pv = pool("ps_pv", 2, space="PSUM")  # 2 x 1-bank slots (pv chunks + nat)

    ident = singles.tile([P, P], F32, tag="ident")
    make_identity(nc, ident)

    three = S_MODE == "bf16_3p"
    sdt = F32 if three else F32R

    def prep_io(b):
        bc = _BatchCtx()
        bc.m_sb = stage.tile([P, nkb], F32, tag="m", name=f"m_sb{b}")
        nc.sync.dma_start(out=bc.m_sb, in_=m_d[b].rearrange("(t p) -> p t", p=P))
        ksrc = k_d[b].rearrange("(t p) d -> p t d", p=P)
        bc.knat = stage.tile([P, nkb, D], F32, tag="knat", name=f"knat{b}")
        h0 = 2 * ((npair + 1) // 2)  # covers the k-pairs of transpose group 0
        nc.gpsimd.dma_start(out=bc.knat[:, :h0, :], in_=ksrc[:, :h0, :])
        nc.gpsimd.dma_start(out=bc.knat[:, h0:, :], in_=ksrc[:, h0:, :])
        bc.qdup = stage.tile([P, NQB, 2, D], F32, tag="qdup", name=f"qdup{b}")
        qsrc = q_d[b].rearrange("(t p) d -> p t d", p=P)
        for g in range(2):
            gs = slice(g * 8, (g + 1) * 8)
            nc.sync.dma_start(out=bc.qdup[:, gs, 0, :], in_=qsrc[:, gs, :])
            nc.sync.dma_start(out=bc.qdup[:, gs, 1, :], in_=qsrc[:, gs, :])
        bc.vnat = stage.tile([P, nkb, D], F32, tag="vnat", name=f"vnat{b}")
        nc.gpsimd.dma_start(out=bc.vnat, in_=v_d[b].rearrange("(t p) d -> p t d", p=P))
        return bc

    def prep_units(b, bc):
        """Closures emitting prep compute; callable in order, spreadable."""
        km = stage.tile([P, nkb, D], F32, tag="km", name=f"km{b}")
        bc.kmT = main.tile([P, npair, P], sdt, tag="kmT", name=f"kmT{b}")
        if three:
            bc.kmTh = main.tile([P, npair, P], BF16, tag="kmTh", name=f"kmTh{b}")
            bc.kmTl = main.tile([P, npair, P], BF16, tag="kmTl", name=f"kmTl{b}")
        bc.qT = main.tile([P, LQ], sdt, tag="qT", name=f"qT{b}")
        if three:
            bc.qTh = main.tile([P, LQ], BF16, tag="qTh", name=f"qTh{b}")
            bc.qTl = main.tile([P, LQ], BF16, tag="qTl", name=f"qTl{b}")
        bc.vme = stage.tile(
            [P, nkb, D + 1], F32R if PV_MODE == "f32r" else F32, tag="vme",
            name=f"vme{b}",
        )
        bc.out_sb = outp.tile([P, NQB, D], F32, tag="osb", name=f"osb{b}")

        h0 = 2 * ((npair + 1) // 2)

        def u_km(half):
            def go():
                for t in range(h0 if half else 0, nkb if half else h0):
                    nc.vector.tensor_scalar_mul(
                        km[:, t, :], bc.knat[:, t, :], bc.m_sb[:, t : t + 1]
                    )
            return go

        def u_kmT(grp):
            jlo = grp * (npair + 1) // 2
            jhi = npair if grp else (npair + 1) // 2
            def go():
                nj = jhi - jlo
                tr = ps_s.tile([P, nj * P], F32, tag="s", name=f"trk{b}_{grp}")
                for j in range(jlo, jhi):
                    nc.tensor.transpose(
                        tr[:, (j - jlo) * P : (j - jlo + 1) * P],
                        km[:, 2 * j : 2 * j + 2, :], ident,
                    )
                dst = bc.kmT[:, jlo:jhi, :].rearrange("p a b -> p (a b)")
                nc.vector.tensor_copy(dst, tr)
                if three:
                    dh = bc.kmTh[:, jlo:jhi, :].rearrange("p a b -> p (a b)")
                    nc.vector.tensor_copy(dh, dst)
                    nc.vector.tensor_sub(
                        bc.kmTl[:, jlo:jhi, :].rearrange("p a b -> p (a b)"),
                        dst, dh,
                    )
            return go

        def u_qT(g):
            def go():
                tr = ps_s.tile([P, 8 * P], F32, tag="s", name=f"trq{b}_{g}")
                for i in range(8):
                    t = g * 8 + i
                    nc.tensor.transpose(
                        tr[:, i * P : (i + 1) * P], bc.qdup[:, t], ident
                    )
                half = slice(g * 8 * P, (g + 1) * 8 * P)
                nc.vector.tensor_copy(bc.qT[:, half], tr)
                if three:
                    nc.vector.tensor_copy(bc.qTh[:, half], bc.qT[:, half])
                    nc.vector.tensor_sub(
                        bc.qTl[:, half], bc.qT[:, half], bc.qTh[:, half]
                    )
            return go

        def u_vme():
            for t in range(nkb):
                nc.vector.tensor_scalar_mul(
                    bc.vme[:, t, 0:D], bc.vnat[:, t, :], bc.m_sb[:, t : t + 1]
                )
            nc.vector.tensor_copy(bc.vme[:, :, D], bc.m_sb[:, :])

        return [u_km(0), u_kmT(0), u_qT(0), u_km(1), u_kmT(1), u_vme, u_qT(1)]

    def main_half(b, bc, h, side_work=()):
        side = list(side_work)
        pvc = [
            ps_pv.tile([65, 512], F32, tag="pv", name=f"pv{b}_{h}_{c}")
            for c in range(2)
        ]
        if three:
            passes = [
                (bc.kmTh, bc.qTh, True, False),
                (bc.kmTl, bc.qTh, False, False),
                (bc.kmTh, bc.qTl, False, True),
            ]
        else:
            passes = [(bc.kmT, bc.qT, True, True)]
        wdt = F32R if PV_MODE == "f32r" else F32

        def emit_pv(j, wA, wB):
            for c in range(2):
                cs = slice(c * 512, (c + 1) * 512)
                for kb, w in ((2 * j, wA), (2 * j + 1, wB)):
                    nc.tensor.matmul(
                        pvc[c], bc.vme[:, kb, :], w[:, cs],
                        start=(kb == 0), stop=(kb == nkb - 1),
                    )

        prev = None
        for j in range(npair):
            sA = ps_s.tile([P, 1024], F32, tag="s", name=f"sA{b}_{h}_{j}")
            sB = ps_s.tile([P, 1024], F32, tag="s", name=f"sB{b}_{h}_{j}")
            # c innermost: consecutive same-side matmuls alternate banks (no
            # accumulate drain-wait) and A/B stay adjacent so they row-pair.
            for kt, qt, st, sp in passes:
                for c in range(2):
                    qs = slice(h * 1024 + c * 512, h * 1024 + (c + 1) * 512)
                    cs = slice(c * 512, (c + 1) * 512)
                    nc.tensor.matmul(
                        sA[:, cs], kt[0:64, j, :], qt[0:64, qs],
                        start=st, stop=sp, tile_position=(0, 0),
                    )
                    nc.tensor.matmul(
                        sB[:, cs], kt[64:128, j, :], qt[64:128, qs],
                        start=st, stop=sp, tile_position=(64, 0),
                    )
            wA = wtp.tile([P, 1024], wdt, tag="wt", name=f"wA{b}_{h}_{j}")
            wB = wtp.tile([P, 1024], wdt, tag="wt", name=f"wB{b}_{h}_{j}")
            nc.scalar.activation(out=wA, in_=sA, func=EXP)
            nc.scalar.activation(out=wB, in_=sB, func=EXP)
            # PV for the previous j: its exp finished while S(j) streamed, so
            # the in-order PE never stalls on ScalarE here.
            if prev is not None:
                emit_pv(prev[0], prev[1], prev[2])
            if side:
                side.pop(0)()
            prev = (j, wA, wB)
        emit_pv(*prev)
        while side:
            side.pop(0)()

        # drain: transpose back to [q, 65] and normalize
        outT = outp.tile([D + 1, 1024], F32, tag="outT", name=f"outT{b}_{h}")
        for c in range(2):
            nc.vector.tensor_copy(outT[:, c * 512 : (c + 1) * 512], pvc[c])
        for qb in range(8):
            nat = ps_pv.tile([P, D + 1], F32, tag="pv", name=f"nat{b}_{h}_{qb}")
            nc.tensor.transpose(
                nat, outT[:, qb * P : (qb + 1) * P], ident[0 : D + 1, 0 : D + 1]
            )
            rc = smalls.tile([P, 1], F32, tag="rc", name=f"rc{b}_{h}_{qb}")
            nc.vector.reciprocal(rc, nat[:, D : D + 1])
            nc.vector.tensor_scalar_mul(
                bc.out_sb[:, h * 8 + qb, :], nat[:, 0:D], rc
            )

    def store(b, bc):
        nc.sync.dma_start(
            out=o_d[b].rearrange("(t p) d -> p t d", p=P), in_=bc.out_sb
        )

    # Interleave batch 1's prep into batch 0's stream: no PE bubble at the
    # batch boundary, and prep transposes spread out so HAM stays warm.
    bcs = [prep_io(b) for b in range(PB)]
    units0 = prep_units(0, bcs[0])
    for u in units0:
        u()
    units1 = prep_units(1, bcs[1]) if PB > 1 else []
    main_half(0, bcs[0], 0, side_work=units1[:4])
    main_half(0, bcs[0], 1, side_work=units1[4:])
    store(0, bcs[0])
    for b in range(1, PB):
        main_half(b, bcs[b], 0)
        main_half(b, bcs[b], 1)
        store(b, bcs[b])

    for p in reversed(pools):
        p.release()


_NC_CACHE = {}


def _build_nc(nkb):
    if nkb in _NC_CACHE:
        return _NC_CACHE[nkb]
    lk = nkb * P
    nc = bacc.Bacc(None, target_bir_lowering=False, debug=False)
    q_d = nc.dram_tensor("q", [PB, LQ, D], F32, kind="ExternalInput")
    k_d = nc.dram_tensor("k", [PB, lk, D], F32, kind="ExternalInput")
    v_d = nc.dram_tensor("v", [PB, lk, D], F32, kind="ExternalInput")
    m_d = nc.dram_tensor("m", [PB, lk], F32, kind="ExternalInput")
    o_d = nc.dram_tensor("out", [PB, LQ, D], F32, kind="ExternalOutput")
    with tile.TileContext(nc) as tc:
        _attention_core(tc, q_d, k_d, v_d, m_d, o_d, nkb)
    nc.compile()
    _NC_CACHE[nkb] = nc
    return nc


def kernel(q, k, v, v_mask, _trace=False, _tmpdir=None):
    q = np.ascontiguousarray(q, dtype=np.float32)
    k = np.ascontiguousarray(k, dtype=np.float32)
    v = np.ascontiguousarray(v, dtype=np.float32)
    v_mask = np.ascontiguousarray(v_mask, dtype=np.float32)
    assert q.shape == (B, LQ, D), q.shape

    counts = (v_mask > 0.5).sum(axis=1)
    if COMPACT and counts.max() <= LKC:
        # kept key rows first (stable), zero-mask padding after; the packed
        # mask makes padded rows contribute exactly 0 on device.
        order = np.argsort(v_mask <= 0.5, axis=1, kind="stable")[:, :LKC]
        kk = np.take_along_axis(k, order[:, :, None], axis=1)
        vv = np.take_along_axis(v, order[:, :, None], axis=1)
        mm = np.take_along_axis(v_mask, order, axis=1)
        nkb = LKC // P
    else:
        kk, vv, mm = k, v, v_mask
        nkb = LK // P

    nc = _build_nc(nkb)
    in_maps = [
        {
            "q": np.ascontiguousarray(q[i * PB : (i + 1) * PB]),
            "k": np.ascontiguousarray(kk[i * PB : (i + 1) * PB]),
            "v": np.ascontiguousarray(vv[i * PB : (i + 1) * PB]),
            "m": np.ascontiguousarray(mm[i * PB : (i + 1) * PB]),
        }
        for i in range(NCORES)
    ]
    res = bass_utils.run_bass_kernel_spmd(
        nc, in_maps, core_ids=list(range(NCORES)), trace=_trace, tmpdir=_tmpdir
    )
    out = np.concatenate([r["out"] for r in res.results], axis=0)
    if _trace:
        kernel.last_results = res
    return out
